# revision 29
# baseline (speedup 1.0000x reference)
"""DYAN encoder (FISTA sparse coding) as a Bass/Tile kernel on 8 trn2 NeuronCores.

Algorithm notes
---------------
reference computes, with D [T=10, K=645] (normalized dictionary), Y = x[0] [10, P]:
    A   = I - D^T D / L,  c = D^T Y / L,  lam = 0.1 / L
    y_0 = x_0 = 0
    for j in 0..99:   (the early-stop never triggers for this data)
        w      = A y_j + c = y_j + (1/L) D^T (Y - D y_j)
        x_{j+1} = softshrink(w, lam)
        y_{j+1} = (1+tt_j) x_{j+1} - tt_j x_j
Since A is I minus a rank-10 term, each iteration only needs thin matmuls:
    u_j = Y - D x_j                    [10, P]   (PE, contraction 645)
    w   = -tt x_{j-1} + (1/L) D^T ((1+tt) u_j - tt u_{j-1}) + (1+tt) x_j
    x_{j+1} = softshrink(w, lam)

Engine split per iteration (P = 1024 per core, two 512-wide PSUM halves):
  - PE:    u matmuls (sy + 6 chunks, x2 halves), rank-20 D^T matmuls
           (6 chunks x2), and the -tt x_{j-1} momentum term for chunks 0..2
           as scaled-identity matmuls (PSUM start=True).
  - Scalar: momentum term for chunks 3..5 as scaled copies SBUF->PSUM
           (activation scale is free), the A/B PSUM->SBUF copies
           ((1+tt)/L u and -tts/L u), and the tiny isc = -tt*I rescale.
  - DVE:   fused softshrink+momentum-add custom op per chunk
           (x_{j+1} = shrink(w_psum + (1+tt) x_j, lam)).
The rank-20 matmuls accumulate on top of the prefilled PSUM (start=False,
skip_group_check for the Scalar-prefilled chunks).

Key scheduling idea: u_{j+1} = Y - D x_{j+1} accumulates INCREMENTALLY
inside iteration j's chunk loop (each chunk's contribution lands right
after its shrink), so the serial u-phase disappears from the recurrence
and the PE stays continuously fed.  The sustained PE rate on this part
is ~0.83 ns/column (mid p-state; the 2.4 GHz boost does not sustain with
all 8 cores active), which makes PE stream-count the controlling budget.

Sharding: pure data parallel over the pixel dim P (8192 -> 8 x 1024).
"""

import os
import numpy as np

T = 10
NDICT = 161
K = 4 * NDICT + 1          # 645
P_FULL = 8192
N_CORES = 8
P = P_FULL // N_CORES      # 1024
NH = 512                   # psum-bank width (fp32)
CH = [128, 128, 128, 128, 128, 5]   # K split into partition chunks
OFF = [0, 128, 256, 384, 512, 640]
NITER = 100
LAMBD = 0.1
PE_ID_CHUNKS = (0, 1, 2)   # chunks whose -tt x_{j-1} prefill rides the PE

_cache = {}


# --------------------------------------------------------------------------- #
# custom DVE ops
# --------------------------------------------------------------------------- #
def _register_dve_op(name, spec):
    import concourse.dve_ops as dve_ops_mod
    from concourse.dve_spec import lower, _has_src1
    from concourse.dve_uop import DveOpSpec

    for o in dve_ops_mod.OPS:
        if o.name == name:
            return o
    row = dve_ops_mod._CUSTOM_DVE_ROW_BASE + len(dve_ops_mod.OPS)
    assert row < 0x20, "DVE opcode rows exhausted"
    shas = {}
    for ver in ("v3", "v4"):
        s = DveOpSpec(name=name, opcode=row, uops=lower(spec, ver=ver),
                      rd1_en=_has_src1(spec))
        shas[ver] = s.sha(ver)
    op = dve_ops_mod.DveOp(name, spec, subdim=False, uops_sha=shas)
    dve_ops_mod.OPS.append(op)
    dve_ops_mod._SUB_OPCODE_FOR_NAME[name] = row
    dve_ops_mod.CUSTOM_DVE_SPECS[name] = spec
    return op


def _get_shrink_op():
    """out = v - clamp(v, -s1, s1) with v = in0 + s0*in1  (softshrink fused
    with the momentum-weighted x add; in0 comes straight from PSUM)."""
    from concourse.dve_spec import Spec, Src0, Src1, C0, C1, C2, maxx, minn

    v = Src0 + C0 * Src1
    body = v - minn(maxx(v, C2), C1)

    def _ref(in0, in1, s0, s1, imm2):
        v = in0.astype(np.float32) + np.float32(s0) * in1.astype(np.float32)
        return v - np.minimum(np.maximum(v, np.float32(imm2)), np.float32(s1))

    return _register_dve_op("FISTA_SHRINK_ANT", Spec(body=body, reference=_ref))


def _get_shrink0_op():
    """out = in0 - clamp(in0, -s1, s1)  (softshrink only; used at iteration 0
    where x_0 = 0 so there is no momentum term)."""
    from concourse.dve_spec import Spec, Src0, C0, C1, maxx, minn

    body = Src0 - minn(maxx(Src0, C0), C1)

    def _ref(in0, in1, s0, s1, imm2):
        v = in0.astype(np.float32)
        return v - np.minimum(np.maximum(v, np.float32(s0)), np.float32(s1))

    return _register_dve_op("FISTA_SHRINK0_ANT", Spec(body=body, reference=_ref))


# --------------------------------------------------------------------------- #
# host-side precompute
# --------------------------------------------------------------------------- #
def _build_dictionary(rr, theta, t):
    i = np.arange(t, dtype=np.float64)[:, None]
    rr = rr.astype(np.float64)
    theta = theta.astype(np.float64)
    rp = rr[None, :] ** i
    sgn = np.where(np.arange(t)[:, None] % 2 == 0, 1.0, -1.0)
    c = np.cos(i * theta[None, :])
    s = np.sin(i * theta[None, :])
    ones = np.ones((t, 1))
    dic = np.concatenate([ones, rp * c, sgn * rp * c, rp * s, sgn * rp * s], axis=1)
    g = np.linalg.norm(dic, axis=0)
    g = np.where(g == 0, np.sqrt(t), g)
    return dic / g


def _momentum_coeffs(n_iter):
    ts = []
    t = 1.0
    for _ in range(n_iter):
        t_new = (1.0 + np.sqrt(1.0 + 4.0 * t * t)) / 2.0
        ts.append((t - 1.0) / t_new)
        t = t_new
    return np.asarray(ts, dtype=np.float32)


# --------------------------------------------------------------------------- #
# device module
# --------------------------------------------------------------------------- #
def _build_module(lam, linv, tts):
    import concourse.bacc as bacc
    import concourse.mybir as mybir
    import concourse.tile as tile

    F32 = mybir.dt.float32
    F32R = mybir.dt.float32r
    shrink_op = _get_shrink_op()
    shrink0_op = _get_shrink0_op()

    nc = bacc.Bacc("TRN2", target_bir_lowering=False, debug=False)

    y_d = nc.dram_tensor("y_in", [T, P], F32R, kind="ExternalInput").ap()
    sy_d = nc.dram_tensor("s_y", [T, 42], F32R, kind="ExternalInput").ap()
    sd_d = nc.dram_tensor("s_d", [K, 42], F32R, kind="ExternalInput").ap()
    wab_d = nc.dram_tensor("w_ab", [42, 768], F32R, kind="ExternalInput").ap()
    z_d = nc.dram_tensor("zeros", [22, P], F32R, kind="ExternalInput").ap()
    i_d = nc.dram_tensor("i_const", [128, 128], F32R, kind="ExternalInput").ap()
    out_d = nc.dram_tensor("out", [K, P], F32, kind="ExternalOutput").ap()

    # per-iteration scalars (fp32-exact python floats)
    tt_prev = [0.0] + [float(tts[j]) for j in range(NITER - 1)]
    lam_f = float(np.float32(lam))
    linv_f = float(np.float32(linv))

    with tile.TileContext(nc) as tc:
        with (
            tc.tile_pool(name="const", bufs=1) as const,
            tc.tile_pool(name="state", bufs=1) as state,
            tc.tile_pool(name="iscp", bufs=2) as iscp,
            tc.tile_pool(name="upool", bufs=1, space="PSUM") as upool,
            tc.tile_pool(name="wpool", bufs=3, space="PSUM") as wpool,
        ):
            y_t = const.tile([T, P], F32R, tag="y", name="y_t")
            sy_t = const.tile([T, 42], F32R, tag="sy", name="sy_t")
            wab_t = const.tile([42, 768], F32R, tag="wab", name="wab_t")
            i_t = const.tile([128, 128], F32R, tag="ic", name="i_t")
            sd_t = [const.tile([CH[c], 42], F32R, tag=f"sd{c}", name=f"sd_t{c}") for c in range(6)]

            nc.sync.dma_start(out=y_t[:], in_=y_d[:])
            nc.sync.dma_start(out=sy_t[:], in_=sy_d[:])
            nc.sync.dma_start(out=wab_t[:], in_=wab_d[:])
            nc.sync.dma_start(out=i_t[:], in_=i_d[:])
            for c in range(6):
                nc.sync.dma_start(out=sd_t[c][:], in_=sd_d[OFF[c]:OFF[c] + CH[c], :])

            xt = [[state.tile([CH[c], P], F32R, tag=f"x{g}_{c}", name=f"x{g}_{c}") for c in range(6)]
                  for g in range(3)]
            ab_ts = [state.tile([42, P], F32R, tag=f"AB{p}", name=f"ab_t{p}")
                     for p in range(2)]
            # rows 10..31 are dead contraction lanes of the merged matmul:
            # must be finite (stationary rows there are zero)
            for p in range(2):
                nc.sync.dma_start(out=ab_ts[p][10:32, :], in_=z_d[:])

            # Iteration specialization (avoids any zero-init):
            #   j=0: x_0 = x_{-1} = 0 -> u_0 = Y (no x-stream), no momentum
            #        prefill, no B_{-1}, plain shrink (no momentum add).
            #   j=1: tt_prev = tts[0] = 0 -> no momentum prefill; the rank-20
            #        matmul runs with B_0 (zeros, since b_scale(0) = 0).
            #
            # Dataflow: u_{j+1} = Y - D x_{j+1} accumulates INCREMENTALLY
            # inside iteration j's chunk loop — chunk c's contribution is
            # emitted two chunks after its shrink so the PE never waits on
            # the DVE.  This removes the serial u-phase from the recurrence.
            u_cur = upool.tile([42, P], F32, tag="u", name="u_ps")
            for h in (0, 1):
                sl = slice(NH * h, NH * (h + 1))
                nc.tensor.matmul(u_cur[:, sl], sy_t[:], y_t[:, sl],
                                 start=True, stop=True)   # u_0 = Y (rep)

            isc = None   # scaled identity for iteration j, emitted at j-1
            for j in range(NITER):
                ttp = tt_prev[j]
                gm1, g0, g1 = (j + 2) % 3, j % 3, (j + 1) % 3
                ab_cur = ab_ts[j % 2]
                ab_next = ab_ts[(j + 1) % 2]
                a_scale = float(np.float32((1.0 + ttp) * linv_f))
                b_scale = float(np.float32(-float(tts[j]) * linv_f))
                pre = ttp != 0.0       # x_{j-1} != 0, i.e. j >= 2
                kc = 42 if j >= 1 else T
                last = j == NITER - 1

                if pre:
                    # scaled identity stationary for the PE momentum chunks
                    isc = iscp.tile([128, 128], F32R, tag="isc", name="isc")
                    nc.scalar.mul(isc[:], i_t[:], float(np.float32(-ttp)))

                # A_j = (1+tt)/L u_j, per half so w c0 h0 starts after A-h0
                for h in (0, 1):
                    sl = slice(NH * h, NH * (h + 1))
                    nc.scalar.mul(ab_cur[0:T, sl], u_cur[0:T, sl], a_scale)
                if not last:
                    nc.scalar.mul(ab_next[32:42, :], u_cur[32:42, :], b_scale)
                    # u_{j+1} reuses the same PSUM bank (WAR on the copies)
                    u_next = upool.tile([42, P], F32, tag="u", name="u_ps")

                for c in range(6):
                    wt = wpool.tile([CH[c], P], F32, tag="w", name=f"w{c}")
                    # momentum prefill: w = -tt * x_{j-1}
                    if pre:
                        if c in PE_ID_CHUNKS:
                            for h in (0, 1):
                                sl = slice(NH * h, NH * (h + 1))
                                nc.tensor.matmul(
                                    wt[:, sl], isc[0:CH[c], 0:CH[c]],
                                    xt[gm1][c][:, sl], start=True, stop=False)
                        else:
                            nc.scalar.mul(wt[:, :], xt[gm1][c][:].bitcast(F32),
                                          float(np.float32(-ttp)))
                    # rank-10+10: w += [D;0;D]^T [A;junk;B]
                    for h in (0, 1):
                        sl = slice(NH * h, NH * (h + 1))
                        nc.tensor.matmul(
                            wt[:, sl],
                            wab_t[0:kc, 128 * c:128 * c + CH[c]],
                            ab_cur[0:kc, sl],
                            start=not pre, stop=True,
                            skip_group_check=pre and c not in PE_ID_CHUNKS)
                    # x_{j+1} = shrink(w + (1+tt) x_j, lam); chunk 0 is
                    # halved so the DVE restarts right after w c0 h0
                    if j == 0:
                        nc.vector._custom_dve(
                            shrink0_op, out=xt[g1][c][:], in0=wt[:],
                            s0=-lam_f, s1=lam_f)
                    elif c == 0:
                        for h in (0, 1):
                            sl = slice(NH * h, NH * (h + 1))
                            nc.vector._custom_dve(
                                shrink_op, out=xt[g1][c][:, sl],
                                in0=wt[:, sl], in1=xt[g0][c][:, sl],
                                s0=float(np.float32(1.0 + ttp)), s1=lam_f,
                                imm2=-lam_f)
                    else:
                        nc.vector._custom_dve(
                            shrink_op, out=xt[g1][c][:], in0=wt[:],
                            in1=xt[g0][c][:],
                            s0=float(np.float32(1.0 + ttp)), s1=lam_f,
                            imm2=-lam_f)
                    if last:
                        nc.sync.dma_start(
                            out=out_d[OFF[c]:OFF[c] + CH[c], :],
                            in_=xt[g1][c][:].bitcast(F32))
                        continue
                    # interleaved u_{j+1} accumulation, lagging 2 chunks
                    if c == 2:
                        for h in (0, 1):
                            sl = slice(NH * h, NH * (h + 1))
                            nc.tensor.matmul(u_next[:, sl], sy_t[:], y_t[:, sl],
                                             start=True, stop=False,
                                             skip_group_check=True)
                    if c >= 2:
                        cc = c - 2
                        for h in (0, 1):
                            sl = slice(NH * h, NH * (h + 1))
                            nc.tensor.matmul(u_next[:, sl], sd_t[cc][:],
                                             xt[g1][cc][:, sl],
                                             start=False, stop=False,
                                             skip_group_check=True)
                if not last:
                    for h in (0, 1):
                        sl = slice(NH * h, NH * (h + 1))
                        for cc in (4, 5):
                            nc.tensor.matmul(u_next[:, sl], sd_t[cc][:],
                                             xt[g1][cc][:, sl],
                                             start=False, stop=(cc == 5),
                                             skip_group_check=True)
                    u_cur = u_next

    nc.compile()
    return nc


# --------------------------------------------------------------------------- #
# entry point
# --------------------------------------------------------------------------- #
def _prepare(x, Drr, Dtheta, t):
    x = np.asarray(x, dtype=np.float32)
    d64 = _build_dictionary(np.asarray(Drr), np.asarray(Dtheta), t)
    dtd = d64.T @ d64
    lspec = np.linalg.norm(dtd, ord=2)
    linv = 1.0 / lspec
    lam = LAMBD * linv
    d32 = d64.astype(np.float32)
    tts = _momentum_coeffs(NITER)

    # u = Y - D x is produced replicated at partition offsets 0 and 32 (the
    # 0-copy feeds the A scaled-copy, the 32-copy feeds the B scaled-copy).
    s_y = np.zeros((T, 42), dtype=np.float32)
    for r in (0, 1):
        s_y[np.arange(T), 32 * r + np.arange(T)] = 1.0
    s_d = np.zeros((K, 42), dtype=np.float32)
    for r in (0, 1):
        s_d[:, 32 * r:32 * r + T] = -d32.T
    # merged rank-20 stationary: rows 0..9 multiply A, rows 32..41 multiply B
    w_ab = np.zeros((42, 768), dtype=np.float32)
    for c in range(6):
        w_ab[0:T, 128 * c:128 * c + CH[c]] = d32[:, OFF[c]:OFF[c] + CH[c]]
        w_ab[32:42, 128 * c:128 * c + CH[c]] = d32[:, OFF[c]:OFF[c] + CH[c]]
    i_const = np.eye(128, dtype=np.float32)
    zeros = np.zeros((22, P), dtype=np.float32)
    return x, lam, linv, tts, s_y, s_d, w_ab, i_const, zeros


def run(x, Drr, Dtheta, T_in, trace=False):
    from concourse.bass_utils import run_bass_kernel_spmd

    t = int(np.asarray(T_in))
    assert t == T
    x, lam, linv, tts, s_y, s_d, w_ab, i_const, zeros = _prepare(x, Drr, Dtheta, t)

    key = ("mod", float(np.float32(lam)), float(np.float32(linv)))
    if key not in _cache:
        _cache[key] = _build_module(lam, linv, tts)
    nc = _cache[key]

    in_maps = []
    for core in range(N_CORES):
        in_maps.append({
            "y_in": np.ascontiguousarray(x[0, :, core * P:(core + 1) * P]),
            "s_y": s_y,
            "s_d": s_d,
            "w_ab": w_ab,
            "i_const": i_const,
            "zeros": zeros,
        })
    res = run_bass_kernel_spmd(nc, in_maps, list(range(N_CORES)), trace=trace)
    out = np.concatenate([res.results[c]["out"] for c in range(N_CORES)], axis=1)
    return out[None, :, :].astype(np.float32), res


def kernel(x, Drr, Dtheta, T, **kw):
    out, _ = run(x, Drr, Dtheta, T, trace=bool(os.environ.get("FISTA_TRACE")))
    return out


# revision 30
# speedup vs baseline: 1.1548x; 1.1548x over previous
"""DYAN encoder (FISTA sparse coding) as a Bass/Tile kernel on 8 trn2 NeuronCores.

Algorithm notes
---------------
reference computes, with D [T=10, K=645] (normalized dictionary), Y = x[0] [10, P]:
    A   = I - D^T D / L,  c = D^T Y / L,  lam = 0.1 / L
    y_0 = x_0 = 0
    for j in 0..99:   (the early-stop never triggers for this data)
        w      = A y_j + c = y_j + (1/L) D^T (Y - D y_j)
        x_{j+1} = softshrink(w, lam)
        y_{j+1} = (1+tt_j) x_{j+1} - tt_j x_j
Since A is I minus a rank-10 term, each iteration only needs thin matmuls:
    u_j = Y - D x_j                    [10, P]   (PE, contraction 645)
    w   = -tt x_{j-1} + (1/L) D^T ((1+tt) u_j - tt u_{j-1}) + (1+tt) x_j
    x_{j+1} = softshrink(w, lam)

Engine split per iteration (P = 1024 per core, two 512-wide PSUM halves):
  - PE:    u matmuls (sy + 6 chunks, x2 halves), rank-20 D^T matmuls
           (6 chunks x2), and the -tt x_{j-1} momentum term for chunks 0..2
           as scaled-identity matmuls (PSUM start=True).
  - Scalar: momentum term for chunks 3..5 as scaled copies SBUF->PSUM
           (activation scale is free), the A/B PSUM->SBUF copies
           ((1+tt)/L u and -tts/L u), and the tiny isc = -tt*I rescale.
  - DVE:   fused softshrink+momentum-add custom op per chunk
           (x_{j+1} = shrink(w_psum + (1+tt) x_j, lam)).
The rank-20 matmuls accumulate on top of the prefilled PSUM (start=False,
skip_group_check for the Scalar-prefilled chunks).

Key scheduling idea: u_{j+1} = Y - D x_{j+1} accumulates INCREMENTALLY
inside iteration j's chunk loop (each chunk's contribution lands right
after its shrink), so the serial u-phase disappears from the recurrence
and the PE stays continuously fed.  The sustained PE rate on this part
is ~0.83 ns/column (mid p-state; the 2.4 GHz boost does not sustain with
all 8 cores active), which makes PE stream-count the controlling budget.

Sharding: pure data parallel over the pixel dim P (8192 -> 8 x 1024).
"""

import os
import numpy as np

T = 10
NDICT = 161
K = 4 * NDICT + 1          # 645
P_FULL = 8192
N_CORES = 8
P = P_FULL // N_CORES      # 1024
NH = 512                   # psum-bank width (fp32)
CH = [128, 128, 128, 128, 128, 5]   # K split into partition chunks
OFF = [0, 128, 256, 384, 512, 640]
NITER = 100
LAMBD = 0.1
PE_ID_CHUNKS = (0, 1, 2)   # chunks whose -tt x_{j-1} prefill rides the PE

_cache = {}


# --------------------------------------------------------------------------- #
# custom DVE ops
# --------------------------------------------------------------------------- #
def _register_dve_op(name, spec):
    import concourse.dve_ops as dve_ops_mod
    from concourse.dve_spec import lower, _has_src1
    from concourse.dve_uop import DveOpSpec

    for o in dve_ops_mod.OPS:
        if o.name == name:
            return o
    row = dve_ops_mod._CUSTOM_DVE_ROW_BASE + len(dve_ops_mod.OPS)
    assert row < 0x20, "DVE opcode rows exhausted"
    shas = {}
    for ver in ("v3", "v4"):
        s = DveOpSpec(name=name, opcode=row, uops=lower(spec, ver=ver),
                      rd1_en=_has_src1(spec))
        shas[ver] = s.sha(ver)
    op = dve_ops_mod.DveOp(name, spec, subdim=False, uops_sha=shas)
    dve_ops_mod.OPS.append(op)
    dve_ops_mod._SUB_OPCODE_FOR_NAME[name] = row
    dve_ops_mod.CUSTOM_DVE_SPECS[name] = spec
    return op


def _get_shrink_op():
    """out = v - clamp(v, -s1, s1) with v = in0 + s0*in1  (softshrink fused
    with the momentum-weighted x add; in0 comes straight from PSUM)."""
    from concourse.dve_spec import Spec, Src0, Src1, C0, C1, C2, maxx, minn

    v = Src0 + C0 * Src1
    body = v - minn(maxx(v, C2), C1)

    def _ref(in0, in1, s0, s1, imm2):
        v = in0.astype(np.float32) + np.float32(s0) * in1.astype(np.float32)
        return v - np.minimum(np.maximum(v, np.float32(imm2)), np.float32(s1))

    return _register_dve_op("FISTA_SHRINK_ANT", Spec(body=body, reference=_ref))


def _get_shrink0_op():
    """out = in0 - clamp(in0, -s1, s1)  (softshrink only; used at iteration 0
    where x_0 = 0 so there is no momentum term)."""
    from concourse.dve_spec import Spec, Src0, C0, C1, maxx, minn

    body = Src0 - minn(maxx(Src0, C0), C1)

    def _ref(in0, in1, s0, s1, imm2):
        v = in0.astype(np.float32)
        return v - np.minimum(np.maximum(v, np.float32(s0)), np.float32(s1))

    return _register_dve_op("FISTA_SHRINK0_ANT", Spec(body=body, reference=_ref))


# --------------------------------------------------------------------------- #
# host-side precompute
# --------------------------------------------------------------------------- #
def _build_dictionary(rr, theta, t):
    i = np.arange(t, dtype=np.float64)[:, None]
    rr = rr.astype(np.float64)
    theta = theta.astype(np.float64)
    rp = rr[None, :] ** i
    sgn = np.where(np.arange(t)[:, None] % 2 == 0, 1.0, -1.0)
    c = np.cos(i * theta[None, :])
    s = np.sin(i * theta[None, :])
    ones = np.ones((t, 1))
    dic = np.concatenate([ones, rp * c, sgn * rp * c, rp * s, sgn * rp * s], axis=1)
    g = np.linalg.norm(dic, axis=0)
    g = np.where(g == 0, np.sqrt(t), g)
    return dic / g


def _momentum_coeffs(n_iter):
    ts = []
    t = 1.0
    for _ in range(n_iter):
        t_new = (1.0 + np.sqrt(1.0 + 4.0 * t * t)) / 2.0
        ts.append((t - 1.0) / t_new)
        t = t_new
    return np.asarray(ts, dtype=np.float32)


# --------------------------------------------------------------------------- #
# device module
# --------------------------------------------------------------------------- #
def _build_module(lam, linv, tts):
    import concourse.bacc as bacc
    import concourse.mybir as mybir
    import concourse.tile as tile

    F32 = mybir.dt.float32
    F32R = mybir.dt.float32r
    shrink_op = _get_shrink_op()
    shrink0_op = _get_shrink0_op()

    nc = bacc.Bacc("TRN2", target_bir_lowering=False, debug=False)

    y_d = nc.dram_tensor("y_in", [T, P], F32R, kind="ExternalInput").ap()
    sy_d = nc.dram_tensor("s_y", [T, 42], F32R, kind="ExternalInput").ap()
    sd_d = nc.dram_tensor("s_d", [K, 42], F32R, kind="ExternalInput").ap()
    wab_d = nc.dram_tensor("w_ab", [42, 768], F32R, kind="ExternalInput").ap()
    z_d = nc.dram_tensor("zeros", [22, P], F32R, kind="ExternalInput").ap()
    i_d = nc.dram_tensor("i_const", [128, 128], F32R, kind="ExternalInput").ap()
    out_d = nc.dram_tensor("out", [K, P], F32, kind="ExternalOutput").ap()

    # per-iteration scalars (fp32-exact python floats)
    tt_prev = [0.0] + [float(tts[j]) for j in range(NITER - 1)]
    lam_f = float(np.float32(lam))
    linv_f = float(np.float32(linv))

    with tile.TileContext(nc) as tc:
        with (
            tc.tile_pool(name="const", bufs=1) as const,
            tc.tile_pool(name="state", bufs=1) as state,
            tc.tile_pool(name="iscp", bufs=2) as iscp,
            tc.tile_pool(name="upool", bufs=1, space="PSUM") as upool,
            tc.tile_pool(name="wpool", bufs=3, space="PSUM") as wpool,
        ):
            y_t = const.tile([T, P], F32R, tag="y", name="y_t")
            sy_t = const.tile([T, 42], F32R, tag="sy", name="sy_t")
            wab_t = const.tile([42, 768], F32R, tag="wab", name="wab_t")
            i_t = const.tile([128, 128], F32R, tag="ic", name="i_t")
            sd_t = [const.tile([CH[c], 42], F32R, tag=f"sd{c}", name=f"sd_t{c}") for c in range(6)]

            nc.sync.dma_start(out=y_t[:], in_=y_d[:])
            nc.sync.dma_start(out=sy_t[:], in_=sy_d[:])
            nc.sync.dma_start(out=wab_t[:], in_=wab_d[:])
            nc.sync.dma_start(out=i_t[:], in_=i_d[:])
            for c in range(6):
                nc.sync.dma_start(out=sd_t[c][:], in_=sd_d[OFF[c]:OFF[c] + CH[c], :])

            xt = [[state.tile([CH[c], P], F32R, tag=f"x{g}_{c}", name=f"x{g}_{c}") for c in range(6)]
                  for g in range(3)]
            ab_ts = [state.tile([42, P], F32R, tag=f"AB{p}", name=f"ab_t{p}")
                     for p in range(2)]
            # rows 10..31 are dead contraction lanes of the merged matmul:
            # must be finite (stationary rows there are zero)
            for p in range(2):
                nc.sync.dma_start(out=ab_ts[p][10:32, :], in_=z_d[:])

            # Iteration specialization (avoids any zero-init):
            #   j=0: x_0 = x_{-1} = 0 -> u_0 = Y (no x-stream), no momentum
            #        prefill, no B_{-1}, plain shrink (no momentum add).
            #   j=1: tt_prev = tts[0] = 0 -> no momentum prefill; the rank-20
            #        matmul runs with B_0 (zeros, since b_scale(0) = 0).
            #
            # Dataflow: u_{j+1} = Y - D x_{j+1} accumulates INCREMENTALLY
            # inside iteration j's chunk loop — chunk c's contribution is
            # emitted two chunks after its shrink so the PE never waits on
            # the DVE.  This removes the serial u-phase from the recurrence.
            u_cur = upool.tile([42, P], F32, tag="u", name="u_ps")
            for h in (0, 1):
                sl = slice(NH * h, NH * (h + 1))
                nc.tensor.matmul(u_cur[:, sl], sy_t[:], y_t[:, sl],
                                 start=True, stop=True)   # u_0 = Y (rep)

            isc = None   # scaled identity for iteration j, emitted at j-1
            for j in range(NITER):
                ttp = tt_prev[j]
                gm1, g0, g1 = (j + 2) % 3, j % 3, (j + 1) % 3
                ab_cur = ab_ts[j % 2]
                ab_next = ab_ts[(j + 1) % 2]
                a_scale = float(np.float32((1.0 + ttp) * linv_f))
                b_scale = float(np.float32(-float(tts[j]) * linv_f))
                pre = ttp != 0.0       # x_{j-1} != 0, i.e. j >= 2
                kc = 42 if j >= 1 else T
                last = j == NITER - 1

                if pre:
                    # scaled identity stationary for the PE momentum chunks
                    isc = iscp.tile([128, 128], F32R, tag="isc", name="isc")
                    nc.scalar.mul(isc[:], i_t[:], float(np.float32(-ttp)))

                # A_j = (1+tt)/L u_j
                nc.scalar.mul(ab_cur[0:T, :], u_cur[0:T, :], a_scale)
                if not last:
                    nc.scalar.mul(ab_next[32:42, :], u_cur[32:42, :], b_scale)
                    # u_{j+1} reuses the same PSUM bank (WAR on the copies)
                    u_next = upool.tile([42, P], F32, tag="u", name="u_ps")

                for c in range(6):
                    wt = wpool.tile([CH[c], P], F32, tag="w", name=f"w{c}")
                    # momentum prefill: w = -tt * x_{j-1}
                    if pre:
                        if c in PE_ID_CHUNKS:
                            for h in (0, 1):
                                sl = slice(NH * h, NH * (h + 1))
                                nc.tensor.matmul(
                                    wt[:, sl], isc[0:CH[c], 0:CH[c]],
                                    xt[gm1][c][:, sl], start=True, stop=False)
                        else:
                            nc.scalar.mul(wt[:, :], xt[gm1][c][:].bitcast(F32),
                                          float(np.float32(-ttp)))
                    # rank-10+10: w += [D;0;D]^T [A;junk;B]
                    for h in (0, 1):
                        sl = slice(NH * h, NH * (h + 1))
                        nc.tensor.matmul(
                            wt[:, sl],
                            wab_t[0:kc, 128 * c:128 * c + CH[c]],
                            ab_cur[0:kc, sl],
                            start=not pre, stop=True,
                            skip_group_check=pre and c not in PE_ID_CHUNKS)
                    # x_{j+1} = shrink(w + (1+tt) x_j, lam)
                    if j == 0:
                        nc.vector._custom_dve(
                            shrink0_op, out=xt[g1][c][:], in0=wt[:],
                            s0=-lam_f, s1=lam_f)
                    else:
                        nc.vector._custom_dve(
                            shrink_op, out=xt[g1][c][:], in0=wt[:],
                            in1=xt[g0][c][:],
                            s0=float(np.float32(1.0 + ttp)), s1=lam_f,
                            imm2=-lam_f)
                    if last:
                        nc.sync.dma_start(
                            out=out_d[OFF[c]:OFF[c] + CH[c], :],
                            in_=xt[g1][c][:].bitcast(F32))
                        continue
                    # interleaved u_{j+1} accumulation, lagging 2 chunks
                    if c == 2:
                        for h in (0, 1):
                            sl = slice(NH * h, NH * (h + 1))
                            nc.tensor.matmul(u_next[:, sl], sy_t[:], y_t[:, sl],
                                             start=True, stop=False,
                                             skip_group_check=True)
                    if c >= 2:
                        cc = c - 2
                        for h in (0, 1):
                            sl = slice(NH * h, NH * (h + 1))
                            nc.tensor.matmul(u_next[:, sl], sd_t[cc][:],
                                             xt[g1][cc][:, sl],
                                             start=False, stop=False,
                                             skip_group_check=True)
                if not last:
                    for cc in (4, 5):
                        for h in (0, 1):
                            sl = slice(NH * h, NH * (h + 1))
                            nc.tensor.matmul(u_next[:, sl], sd_t[cc][:],
                                             xt[g1][cc][:, sl],
                                             start=False, stop=(cc == 5),
                                             skip_group_check=True)
                    u_cur = u_next

    nc.compile()
    return nc


# --------------------------------------------------------------------------- #
# entry point
# --------------------------------------------------------------------------- #
def _prepare(x, Drr, Dtheta, t):
    x = np.asarray(x, dtype=np.float32)
    d64 = _build_dictionary(np.asarray(Drr), np.asarray(Dtheta), t)
    dtd = d64.T @ d64
    lspec = np.linalg.norm(dtd, ord=2)
    linv = 1.0 / lspec
    lam = LAMBD * linv
    d32 = d64.astype(np.float32)
    tts = _momentum_coeffs(NITER)

    # u = Y - D x is produced replicated at partition offsets 0 and 32 (the
    # 0-copy feeds the A scaled-copy, the 32-copy feeds the B scaled-copy).
    s_y = np.zeros((T, 42), dtype=np.float32)
    for r in (0, 1):
        s_y[np.arange(T), 32 * r + np.arange(T)] = 1.0
    s_d = np.zeros((K, 42), dtype=np.float32)
    for r in (0, 1):
        s_d[:, 32 * r:32 * r + T] = -d32.T
    # merged rank-20 stationary: rows 0..9 multiply A, rows 32..41 multiply B
    w_ab = np.zeros((42, 768), dtype=np.float32)
    for c in range(6):
        w_ab[0:T, 128 * c:128 * c + CH[c]] = d32[:, OFF[c]:OFF[c] + CH[c]]
        w_ab[32:42, 128 * c:128 * c + CH[c]] = d32[:, OFF[c]:OFF[c] + CH[c]]
    i_const = np.eye(128, dtype=np.float32)
    zeros = np.zeros((22, P), dtype=np.float32)
    return x, lam, linv, tts, s_y, s_d, w_ab, i_const, zeros


def run(x, Drr, Dtheta, T_in, trace=False):
    from concourse.bass_utils import run_bass_kernel_spmd

    t = int(np.asarray(T_in))
    assert t == T
    x, lam, linv, tts, s_y, s_d, w_ab, i_const, zeros = _prepare(x, Drr, Dtheta, t)

    key = ("mod", float(np.float32(lam)), float(np.float32(linv)))
    if key not in _cache:
        _cache[key] = _build_module(lam, linv, tts)
    nc = _cache[key]

    in_maps = []
    for core in range(N_CORES):
        in_maps.append({
            "y_in": np.ascontiguousarray(x[0, :, core * P:(core + 1) * P]),
            "s_y": s_y,
            "s_d": s_d,
            "w_ab": w_ab,
            "i_const": i_const,
            "zeros": zeros,
        })
    res = run_bass_kernel_spmd(nc, in_maps, list(range(N_CORES)), trace=trace)
    out = np.concatenate([res.results[c]["out"] for c in range(N_CORES)], axis=1)
    return out[None, :, :].astype(np.float32), res


def kernel(x, Drr, Dtheta, T, **kw):
    out, _ = run(x, Drr, Dtheta, T, trace=bool(os.environ.get("FISTA_TRACE")))
    return out
